# revision 11
# baseline (speedup 1.0000x reference)
"""Trainium2 Bass kernel for nn_DeepMDsimpleEnergy.

Math: with zero MLP biases and strictly-positive scalar MLP inputs (r and 1/r),
each relu-pyramid MLP collapses exactly to a linear map: mlp(x) = v * x with
v = fold(max(.,0) of cascaded weights).  Hence
    L1 = mlp_r(r) * r     = v_r * r^2
    L2 = mlp_i(1/r) * 1/r = v_i * (1/r)^2
    D  = [v_r * a, v_i * b],  a = sum_j r_j^2, b = sum_j r_j^-2  (23 neighbors)
and the fit MLP's first layer becomes relu(a*u1 + b*u2 + b1) with
u1 = v_r @ W1[:32], u2 = v_i @ W1[32:].  The device computes, per pair row,
d = own - neighbor, d^2 and 1/d^2, segment-sums them with two indicator
matmuls, and runs the tiny fit MLP with block-diagonal (8-sample-group)
matmuls.  Sharded data-parallel over samples: 8 samples per NeuronCore.

Self-pairs are kept (d = 0): they add exactly 0 to `a`, and for `b` the mask
M01*1e30 makes 1/d^2 ~ 1e-30 (negligible, far below fp32 rounding of b).
"""

import numpy as np

import concourse.bacc as bacc
import concourse.bass as bass
import concourse.tile as tile
from concourse import mybir
from concourse.bass_utils import run_bass_kernel_spmd

F32 = mybir.dt.float32
NS, NCELLS, NP = 64, 64, 8
NCORES = 8
SPC = 8          # samples per core
SLAB = 4         # samples per d-tile slab (2 slabs per core)
NPART = 96       # partitions used by pair tiles: 24 neighbors x 4 samples
NFREE = 512      # free dim: p*64 + c  (particles of one sample)
BIG = 1.0e30

_prog_cache = {}


def _dram_ap(handle, offset, dims):
    base = handle[:]
    return bass.AP(tensor=base.tensor, offset=offset, ap=[list(d) for d in dims])


def _build_program():
    if "nc" in _prog_cache:
        return _prog_cache["nc"]

    nc = bacc.Bacc(
        "TRN2", target_bir_lowering=False, debug=False, enable_asserts=False
    )

    r_ext = nc.dram_tensor("R_ext", [SPC, 528], F32, kind="ExternalInput")
    m01 = nc.dram_tensor("M01", [NPART, NP], F32, kind="ExternalInput")
    l_all = nc.dram_tensor("L_all", [12, 9 * NPART], F32, kind="ExternalInput")
    s_mats = [
        nc.dram_tensor(f"S{i}", [NPART, 16], F32, kind="ExternalInput")
        for i in range(4)
    ]  # S0=S_r slab0, S1=S_i slab0, S2=S_r slab1, S3=S_i slab1
    fit_dims = [(16, 128), (128, 64), (64, 32), (32, 16), (16, 8)]
    w_fit = [
        nc.dram_tensor(f"W{i}p", [k, m], F32, kind="ExternalInput")
        for i, (k, m) in enumerate(fit_dims)
    ]
    b_fit = [
        nc.dram_tensor(f"b{i}v", [m, 1], F32, kind="ExternalInput")
        for i, (k, m) in enumerate(fit_dims)
    ]
    lin_wv = nc.dram_tensor("linWv", [SPC, 1], F32, kind="ExternalInput")
    lin_bv = nc.dram_tensor("linbv", [SPC, 1], F32, kind="ExternalInput")
    e_out = nc.dram_tensor("E_out", [SPC, 1], F32, kind="ExternalOutput")

    with tile.TileContext(nc) as tc:
        with (
            tc.tile_pool(name="consts", bufs=1) as consts,
            tc.tile_pool(name="work", bufs=2) as work,
            tc.tile_pool(name="fit", bufs=1) as fit,
            tc.tile_pool(name="pseg", bufs=1, space="PSUM") as pseg,
            tc.tile_pool(name="pdp", bufs=2, space="PSUM") as pdp,
            tc.tile_pool(name="pfit", bufs=1, space="PSUM") as pfit,
        ):
            m01_sb = consts.tile([NPART, NP], F32)
            nc.sync.dma_start(out=m01_sb[:], in_=m01[:])
            l_sb = consts.tile([12, 9 * NPART], F32)
            nc.sync.dma_start(out=l_sb[:], in_=l_all[:])
            s_sb = []
            for i in range(4):
                t = consts.tile([NPART, 16], F32, tag=f"s{i}")
                nc.sync.dma_start(out=t[:], in_=s_mats[i][:])
                s_sb.append(t)
            w_sb = []
            b_sb = []
            for i, (k, m) in enumerate(fit_dims):
                wt = consts.tile([k, m], F32, tag=f"w{i}")
                nc.sync.dma_start(out=wt[:], in_=w_fit[i][:])
                w_sb.append(wt)
                bt = consts.tile([m, 1], F32, tag=f"bb{i}")
                nc.sync.dma_start(out=bt[:], in_=b_fit[i][:])
                b_sb.append(bt)
            linw_sb = consts.tile([SPC, 1], F32)
            nc.sync.dma_start(out=linw_sb[:], in_=lin_wv[:])
            linb_sb = consts.tile([SPC, 1], F32)
            nc.sync.dma_start(out=linb_sb[:], in_=lin_bv[:])

            psum_seg = pseg.tile([16, NFREE], F32)

            n_mm = 0
            for t in range(2):
                # window tile: w[3*s''+dc, m] = R_ext[sl, 8*dc + m]
                w_t = work.tile([12, NFREE], F32, tag="w")
                nc.sync.dma_start(
                    out=w_t[:],
                    in_=_dram_ap(
                        r_ext, t * SLAB * 528, [(528, SLAB), (NP, 3), (1, NFREE)]
                    ),
                )
                # d[q, n] = A - Y via 9 accumulating indicator matmuls:
                # A[q, 8c+p] = w[3s''+1, 8c+p];  Y[q, 8c+p] = w[3s''+dc, 8c+jm]
                psum_d = pdp.tile([NPART, NFREE], F32, tag="pd")
                wb = w_t[:]
                for i in range(9):
                    if i == 0:
                        rhs = wb  # A term, contiguous
                    else:
                        jm = i - 1
                        rhs = bass.AP(
                            tensor=wb.tensor,
                            offset=wb.offset + jm,
                            ap=[wb.ap[0], [NP, NCELLS], [0, NP]],
                        )
                    nc.tensor.matmul(
                        psum_d[:],
                        lhsT=l_sb[:, NPART * i : NPART * (i + 1)],
                        rhs=rhs,
                        start=(i == 0),
                        stop=(i == 8),
                    )
                x_r = work.tile([NPART, NFREE], F32, tag="xr")
                nc.scalar.square(out=x_r[:], in_=psum_d[:])
                # X_m = M01*BIG + X_r   (self positions -> ~1e30)
                m01_b = m01_sb[:]
                m01_view = bass.AP(
                    tensor=m01_b.tensor,
                    offset=m01_b.offset,
                    ap=[m01_b.ap[0], [0, NCELLS], [1, NP]],
                )
                x_m = work.tile([NPART, NFREE], F32, tag="xm")
                nc.vector.scalar_tensor_tensor(
                    out=x_m[:],
                    in0=m01_view,
                    scalar=BIG,
                    in1=x_r[:],
                    op0=mybir.AluOpType.mult,
                    op1=mybir.AluOpType.add,
                )
                x_i = work.tile([NPART, NFREE], F32, tag="xi")
                nc.vector.reciprocal(out=x_i[:], in_=x_m[:])

                for s_mat, x_t in ((s_sb[2 * t], x_r), (s_sb[2 * t + 1], x_i)):
                    nc.tensor.matmul(
                        psum_seg[:],
                        lhsT=s_mat[:],
                        rhs=x_t[:],
                        start=(n_mm == 0),
                        stop=(n_mm == 3),
                    )
                    n_mm += 1

            h = fit.tile([16, NFREE], F32, tag="fitrhs")
            nc.scalar.copy(out=h[:], in_=psum_seg[:])
            for i, (k, m) in enumerate(fit_dims):
                pm = pfit.tile([m, NFREE], F32, tag=f"pf{i}")
                nc.tensor.matmul(pm[:], lhsT=w_sb[i][:], rhs=h[:], start=True, stop=True)
                h = fit.tile([m, NFREE], F32, tag=f"h{i}")
                if i < len(fit_dims) - 1:
                    nc.scalar.activation(
                        out=h[:],
                        in_=pm[:],
                        func=mybir.ActivationFunctionType.Relu,
                        bias=b_sb[i][:],
                    )
                else:
                    esum = fit.tile([SPC, 1], F32, tag="esum")
                    nc.scalar.activation(
                        out=h[:],
                        in_=pm[:],
                        func=mybir.ActivationFunctionType.Relu,
                        bias=b_sb[i][:],
                        accum_out=esum[:],
                    )
            e_sb = fit.tile([SPC, 1], F32, tag="e")
            nc.scalar.activation(
                out=e_sb[:],
                in_=esum[:],
                func=mybir.ActivationFunctionType.Identity,
                bias=linb_sb[:],
                scale=linw_sb[:],
            )
            nc.sync.dma_start(out=e_out[:], in_=e_sb[:])

    nc.compile()
    _prog_cache["nc"] = nc
    return nc


def _collapse_mlp(params):
    v = np.ones((1,), np.float64)
    for W, b in params:
        W = np.asarray(W, np.float64)
        assert np.all(np.asarray(b) == 0.0), "nonzero pyramid bias breaks collapse"
        v = np.maximum(v @ W, 0.0)
    return v  # mlp(x) = v * x for x > 0


def make_inputs(R, pyr_params, pyr_inv_params, fit_params, lin_W, lin_b, av, std):
    """Host-side preprocessing: returns per-core in_maps."""
    R = np.asarray(R, np.float32)
    av = np.asarray(av, np.float32)
    std = np.asarray(std, np.float32)
    assert np.all(av == 0.0) and np.all(std == 1.0), "av/std folding assumes 0/1"

    v_r = _collapse_mlp(pyr_params)  # (32,)
    v_i = _collapse_mlp(pyr_inv_params)
    fit_np = [(np.asarray(W, np.float64), np.asarray(b, np.float64)) for W, b in fit_params]
    W1 = fit_np[0][0]  # (64,16)
    u1 = v_r @ W1[:32]
    u2 = v_i @ W1[32:]

    # block-diagonal fit weights, 8 sample-groups
    fit_dims = [(16, 128), (128, 64), (64, 32), (32, 16), (16, 8)]
    w_blk = []
    b_blk = []
    mats = [np.stack([u1, u2], axis=0)] + [W for W, _ in fit_np[1:]]
    biases = [b for _, b in fit_np]
    for i, (kd, md) in enumerate(fit_dims):
        kg, mg = kd // 8, md // 8
        Wp = np.zeros((kd, md), np.float32)
        for g in range(8):
            Wp[kg * g : kg * (g + 1), mg * g : mg * (g + 1)] = mats[i]
        w_blk.append(Wp)
        b_blk.append(np.tile(np.asarray(biases[i], np.float32), 8)[:, None])

    # segment-sum indicator matrices
    s_mats = []
    for t in range(2):
        for e in range(2):  # 0 -> X_r (a), 1 -> X_i (b)
            S = np.zeros((NPART, 16), np.float32)
            for s2 in range(SLAB):
                S[24 * s2 : 24 * (s2 + 1), 2 * (SLAB * t + s2) + e] = 1.0
            s_mats.append(S)

    # self mask (p-periodic): q = 24*s'' + 8*dc + jm, col p: 1 iff dc==1, jm==p
    m01 = np.zeros((NPART, NP), np.float32)
    for s2 in range(SLAB):
        for p in range(NP):
            m01[24 * s2 + 8 + p, p] = 1.0

    # indicator lhsT blocks for d = A - Y: L_all[:, 96*i:96*(i+1)]
    # i=0: A:  L[3s''+1, 24s''+j] = +1 for all j
    # i=1+jm: L[3s''+dc, 24s''+8dc+jm] = -1
    l_all = np.zeros((12, 9 * NPART), np.float32)
    for s2 in range(SLAB):
        for j in range(24):
            l_all[3 * s2 + 1, 24 * s2 + j] = 1.0
        for dc in range(3):
            for jm in range(NP):
                l_all[3 * s2 + dc, NPART * (1 + jm) + 24 * s2 + 8 * dc + jm] = -1.0

    lin_w_val = float(np.asarray(lin_W).reshape(-1)[0])
    lin_b_val = float(np.asarray(lin_b).reshape(-1)[0])
    lin_wv = np.full((SPC, 1), lin_w_val, np.float32)
    lin_bv = np.full((SPC, 1), lin_b_val * NCELLS * NP, np.float32)

    in_maps = []
    for core in range(NCORES):
        Rc = R[core * SPC : (core + 1) * SPC]  # (8, 512)
        r_ext = np.concatenate(
            [Rc[:, -8:] - float(NCELLS), Rc, Rc[:, :8] + float(NCELLS)], axis=1
        ).astype(np.float32)  # (8, 528)
        im = {
            "R_ext": r_ext,
            "M01": m01,
            "L_all": l_all,
            "linWv": lin_wv,
            "linbv": lin_bv,
        }
        for i in range(4):
            im[f"S{i}"] = s_mats[i]
        for i in range(5):
            im[f"W{i}p"] = w_blk[i]
            im[f"b{i}v"] = b_blk[i]
        in_maps.append(im)
    return in_maps


def kernel(R, pyr_params, pyr_inv_params, fit_params, lin_W, lin_b, av, std):
    in_maps = make_inputs(
        R, pyr_params, pyr_inv_params, fit_params, lin_W, lin_b, av, std
    )
    nc = _build_program()
    res = run_bass_kernel_spmd(nc, in_maps, core_ids=list(range(NCORES)))
    return np.concatenate(
        [res.results[c]["E_out"] for c in range(NCORES)], axis=0
    ).astype(np.float32)


# revision 29
# speedup vs baseline: 27.2864x; 27.2864x over previous
"""Trainium2 Bass kernel for nn_DeepMDsimpleEnergy.

Math: with zero MLP biases and strictly-positive scalar MLP inputs (r and 1/r),
each relu-pyramid MLP collapses exactly to a linear map: mlp(x) = v * x with
v = fold(max(.,0) of cascaded weights).  Hence
    L1 = mlp_r(r) * r     = v_r * r^2
    L2 = mlp_i(1/r) * 1/r = v_i * (1/r)^2
    D  = [v_r * a, v_i * b],  a = sum_j r_j^2, b = sum_j r_j^-2  (23 neighbors)
and the fit MLP's first layer becomes relu(a*u1 + b*u2 + b1) with
u1 = v_r @ W1[:32], u2 = v_i @ W1[32:].

Device pipeline (per core = 8 samples, two 4-sample slabs):
  - W8 tile [96,512]: 8 jm-shifted copies of each sample's 3-cell window
  - d = A - Y in PSUM via 2 indicator matmuls (A: plain rhs; Y: stride-0
    broadcast rhs covering all 24 neighbors x 512 particles)
  - X_r = square(d) (ACT), X_m = X_r + BIG*selfmask (DVE), X_i = 1/X_m (DVE)
  - a,b per particle via indicator matmuls accumulating into one PSUM tile
  - tiny fit MLP (2->16->8->4->2->1) as block-diagonal matmuls over
    8 sample-groups, relu+bias fused into ACT evictions, final energy via
    ACT accum_out + affine with lin_W/lin_b.
Self-pairs (d=0) add exactly 0 to `a`; the BIG mask makes their 1/d^2
~1e-30, far below fp32 rounding of `b`.
"""

import numpy as np

import concourse.bacc as bacc
import concourse.bass as bass
import concourse.tile as tile
from concourse import mybir
from concourse.bass_utils import run_bass_kernel_spmd

F32 = mybir.dt.float32
NS, NCELLS, NP = 64, 64, 8
NCORES = 8
SPC = 8          # samples per core
SLAB = 4         # samples per slab (2 slabs per core)
NPART = 96       # partitions of pair tiles: 24 neighbors x 4 samples
NFREE = 512      # particles of one sample: n = 8c + p
BIG = 1.0e30
FIT_DIMS = [(16, 128), (128, 64), (64, 32), (32, 16), (16, 8)]

_prog_cache = {}


def _ap(handle_or_ap, offset, dims):
    base = handle_or_ap[:] if hasattr(handle_or_ap, "dtype") is False else handle_or_ap
    if not isinstance(base, bass.AP):
        base = handle_or_ap[:]
    return bass.AP(
        tensor=base.tensor, offset=base.offset + offset, ap=[list(d) for d in dims]
    )


def _build_program():
    if "nc" in _prog_cache:
        return _prog_cache["nc"]

    nc = bacc.Bacc(
        "TRN2", target_bir_lowering=False, debug=False, enable_asserts=False
    )

    r_ext = nc.dram_tensor("R_ext", [SPC, 544], F32, kind="ExternalInput")
    # compact window: R_cmp[sl, w*64 + c] = R_ext[sl, w + 8c]
    r_cmp = nc.dram_tensor("R_cmp", [SPC, 24 * 64], F32, kind="ExternalInput")
    # indicator lhsT for the A-term
    l_a = nc.dram_tensor("L_A", [SLAB, NPART], F32, kind="ExternalInput")
    # P96 [96, 168] = L_Y(-I, 96) | S indicators(64) | M01(8)
    p96 = nc.dram_tensor("P96", [NPART, 168], F32, kind="ExternalInput")
    # P128 [128, 255] = fit weights W2(64) W1(128) W3(32) W4(16) W5(8) | b1..b5 linW linb (7)
    p128 = nc.dram_tensor("P128", [128, 255], F32, kind="ExternalInput")
    e_out = nc.dram_tensor("E_out", [SPC, 1], F32, kind="ExternalOutput")

    dma_engines = [nc.sync, nc.gpsimd, nc.scalar]

    with tile.TileContext(nc) as tc:
        with (
            tc.tile_pool(name="consts", bufs=1) as consts,
            tc.tile_pool(name="work", bufs=2) as work,
            tc.tile_pool(name="fit", bufs=1) as fit,
            tc.tile_pool(name="pseg", bufs=1, space="PSUM") as pseg,
            tc.tile_pool(name="pdp", bufs=2, space="PSUM") as pdp,
            tc.tile_pool(name="pfit", bufs=1, space="PSUM") as pfit,
        ):
            # PE warm-up first: keep PE continuously busy through the DMA
            # preamble so the HAM/pstate ramp completes before real matmuls.
            warm = consts.tile([1, 64], F32)
            nc.vector.memset(warm[:], 1.0)
            pwarm = pseg.tile([1, 64], F32, tag="sg0")
            for _ in range(18):
                nc.tensor.matmul(
                    pwarm[:], lhsT=warm[0:1, 0:1], rhs=warm[:], start=True, stop=True
                )

            # ---- stage 1 inputs first (critical path) ----
            # W8c[24*s'' + w, c] = R_ext[sl, w + 8c]; Ra[s''] = own positions
            w8 = []
            ras = []
            for t in range(2):
                wt = work.tile([NPART, 64], F32, tag=f"w8_{t}")
                dma_engines[t % 3].dma_start(
                    out=wt[:],
                    in_=_ap(r_cmp, t * SLAB * 1536, [(1536, SLAB), (1, 1536)]),
                )
                w8.append(wt)
                ra = work.tile([SLAB, NFREE], F32, tag=f"ra_{t}")
                dma_engines[(t + 2) % 3].dma_start(
                    out=ra[:],
                    in_=_ap(r_ext, t * SLAB * 544 + 8, [(544, SLAB), (1, NFREE)]),
                )
                ras.append(ra)
            la_sb = consts.tile([SLAB, NPART], F32)
            nc.scalar.dma_start(out=la_sb[:], in_=l_a[:])
            p96_sb = consts.tile([NPART, 168], F32)
            nc.sync.dma_start(out=p96_sb[:], in_=p96[:])
            p128_sb = consts.tile([128, 255], F32)
            nc.gpsimd.dma_start(out=p128_sb[:], in_=p128[:])
            ly_sb = p96_sb[:, 0:NPART]
            s_sb = p96_sb[:, NPART : NPART + 64]
            m01_sb = p96_sb[:, NPART + 64 : NPART + 72]
            wpk_sb = p128_sb
            w_sl = {
                "W2": wpk_sb[:, 0:64],
                "W1": wpk_sb[:16, 64:192],
                "W3": wpk_sb[:64, 192:224],
                "W4": wpk_sb[:32, 224:240],
                "W5": wpk_sb[:16, 240:248],
            }
            w_fit = [w_sl["W1"], w_sl["W2"], w_sl["W3"], w_sl["W4"], w_sl["W5"]]
            b_fit = [
                p128_sb[: FIT_DIMS[i][1], 248 + i : 249 + i] for i in range(5)
            ]
            linw_sl = p128_sb[:SPC, 253:254]
            linb_sl = p128_sb[:SPC, 254:255]

            # ---- d = A - Y (both slabs), then elementwise ----
            xs = []  # (x_r, x_i) per slab
            for t in range(2):
                wb = w8[t][:]
                psum_d = pdp.tile([NPART, NFREE], F32, tag="pd")
                nc.tensor.matmul(
                    psum_d[:],
                    lhsT=la_sb[:],
                    rhs=ras[t][:],
                    start=True,
                    stop=False,
                )
                rhs_b = bass.AP(
                    tensor=wb.tensor,
                    offset=wb.offset,
                    ap=[wb.ap[0], [1, NCELLS], [0, NP]],
                )
                nc.tensor.matmul(
                    psum_d[:],
                    lhsT=ly_sb,
                    rhs=rhs_b,
                    start=False,
                    stop=True,
                )
                x_r = work.tile([NPART, NFREE], F32, tag="xr")
                nc.scalar.square(out=x_r[:], in_=psum_d[:])
                m01_b = m01_sb
                m01_view = bass.AP(
                    tensor=m01_b.tensor,
                    offset=m01_b.offset,
                    ap=[m01_b.ap[0], [0, NCELLS], [1, NP]],
                )
                x_m = work.tile([NPART, NFREE], F32, tag="xm")
                nc.gpsimd.tensor_add(out=x_m[:], in0=m01_view, in1=x_r[:])
                x_i = work.tile([NPART, NFREE], F32, tag="xi")
                nc.vector.reciprocal_approx_fast(out=x_i[:], in_=x_m[:])
                xs.append((x_r, x_i))

            # ---- segment sums, split into column halves (separate PSUM
            # banks) so the fit chain for half 0 overlaps half-1 matmuls ----
            order = [(0, 0), (1, 0), (0, 1), (1, 1)]  # (slab, e)
            psum_seg_h = []
            for hf in range(2):
                cs = slice(256 * hf, 256 * (hf + 1))
                ps = pseg.tile([16, 256], F32, tag=f"sg{hf}")
                for n_mm, (t, e) in enumerate(order):
                    nc.tensor.matmul(
                        ps[:],
                        lhsT=s_sb[:, 16 * (2 * t + e) : 16 * (2 * t + e + 1)],
                        rhs=xs[t][e][:, cs],
                        start=(n_mm == 0),
                        stop=(n_mm == 3),
                    )
                psum_seg_h.append(ps)

            # ---- fit MLP, four N=128 quarters pipelined ----
            NQ = 4
            QW = NFREE // NQ
            esums = []
            for hf in range(NQ):
                cs = slice(QW * hf, QW * (hf + 1))
                h = fit.tile([16, QW], F32, tag=f"fr{hf}")
                nc.scalar.copy(
                    out=h[:],
                    in_=psum_seg_h[hf // 2][:, 128 * (hf % 2) : 128 * (hf % 2 + 1)],
                )
                for i, (kd, md) in enumerate(FIT_DIMS):
                    pm = pfit.tile([md, QW], F32, tag=f"pf{hf}")
                    nc.tensor.matmul(
                        pm[:], lhsT=w_fit[i], rhs=h[:], start=True, stop=True
                    )
                    h = fit.tile([md, QW], F32, tag=f"h{i}{hf}")
                    if i < 4:
                        if hf % 2 == 0:
                            nc.scalar.activation(
                                out=h[:],
                                in_=pm[:],
                                func=mybir.ActivationFunctionType.Relu,
                                bias=b_fit[i],
                            )
                        else:
                            nc.vector.tensor_relu(out=h[:], in_=pm[:])
                    else:
                        es = fit.tile([SPC, 1], F32, tag=f"es{hf}")
                        nc.scalar.activation(
                            out=h[:],
                            in_=pm[:],
                            func=mybir.ActivationFunctionType.Relu,
                            bias=b_fit[i],
                            accum_out=es[:],
                        )
                        esums.append(es)
            e01 = fit.tile([SPC, 1], F32, tag="e01")
            nc.vector.tensor_add(out=e01[:], in0=esums[0][:], in1=esums[1][:])
            e23 = fit.tile([SPC, 1], F32, tag="e23")
            nc.vector.tensor_add(out=e23[:], in0=esums[2][:], in1=esums[3][:])
            etot = fit.tile([SPC, 1], F32, tag="etot")
            nc.vector.tensor_add(out=etot[:], in0=e01[:], in1=e23[:])
            e_sb = fit.tile([SPC, 1], F32, tag="e")
            nc.scalar.activation(
                out=e_sb[:],
                in_=etot[:],
                func=mybir.ActivationFunctionType.Identity,
                bias=linb_sl,
                scale=linw_sl,
            )
            nc.sync.dma_start(out=e_out[:], in_=e_sb[:])

    nc.compile()
    _prog_cache["nc"] = nc
    return nc


def _collapse_mlp(params):
    v = np.ones((1,), np.float64)
    for W, b in params:
        W = np.asarray(W, np.float64)
        assert np.all(np.asarray(b) == 0.0), "nonzero pyramid bias breaks collapse"
        v = np.maximum(v @ W, 0.0)
    return v  # mlp(x) = v * x for x > 0


def make_inputs(R, pyr_params, pyr_inv_params, fit_params, lin_W, lin_b, av, std):
    """Host-side preprocessing: returns per-core in_maps."""
    R = np.asarray(R, np.float32)
    av = np.asarray(av, np.float32)
    std = np.asarray(std, np.float32)
    assert np.all(av == 0.0) and np.all(std == 1.0), "av/std folding assumes 0/1"

    v_r = _collapse_mlp(pyr_params)  # (32,)
    v_i = _collapse_mlp(pyr_inv_params)
    fit_np = [
        (np.asarray(W, np.float64), np.asarray(b, np.float64)) for W, b in fit_params
    ]
    for _, fb in fit_np:
        assert np.all(fb == 0.0), "nonzero fit bias: DVE relu eviction drops bias"
    u1 = v_r @ fit_np[0][0][:32]  # (16,)
    u2 = v_i @ fit_np[0][0][32:]

    # block-diagonal fit weights, 8 sample-groups
    mats = [np.stack([u1, u2], axis=0)] + [W for W, _ in fit_np[1:]]
    w_blk, b_vecs = [], []
    for i, (kd, md) in enumerate(FIT_DIMS):
        kg, mg = kd // 8, md // 8
        Wp = np.zeros((kd, md), np.float32)
        for g in range(8):
            Wp[kg * g : kg * (g + 1), mg * g : mg * (g + 1)] = mats[i]
        w_blk.append(Wp)
        b_vecs.append(np.tile(np.asarray(fit_np[i][1], np.float32), 8))

    p128 = np.zeros((128, 255), np.float32)
    p128[:128, 0:64] = w_blk[1]
    p128[:16, 64:192] = w_blk[0]
    p128[:64, 192:224] = w_blk[2]
    p128[:32, 224:240] = w_blk[3]
    p128[:16, 240:248] = w_blk[4]
    lin_w_val = float(np.asarray(lin_W).reshape(-1)[0])
    lin_b_val = float(np.asarray(lin_b).reshape(-1)[0])
    for i in range(5):
        p128[: FIT_DIMS[i][1], 248 + i] = b_vecs[i]
    p128[:SPC, 253] = lin_w_val
    p128[:SPC, 254] = lin_b_val * NCELLS * NP

    # indicator lhsT blocks: A from Ra rows (K=4), Y = -I on W8c rows
    l_a = np.zeros((SLAB, NPART), np.float32)
    for s2 in range(SLAB):
        l_a[s2, 24 * s2 : 24 * (s2 + 1)] = 1.0
    l_y = -np.eye(NPART, dtype=np.float32)

    # P96 = L_Y(-I) | segment indicators | self mask (BIG at self)
    p96 = np.zeros((NPART, 168), np.float32)
    p96[:, :NPART] = l_y
    for t in range(2):
        for e in range(2):
            for s2 in range(SLAB):
                col = NPART + 16 * (2 * t + e) + 2 * (SLAB * t + s2) + e
                p96[24 * s2 : 24 * (s2 + 1), col] = 1.0
    for s2 in range(SLAB):
        for p in range(NP):
            p96[24 * s2 + 8 + p, NPART + 64 + p] = BIG

    in_maps = []
    for core in range(NCORES):
        Rc = R[core * SPC : (core + 1) * SPC]
        r_ext = np.concatenate(
            [Rc[:, -8:] - float(NCELLS), Rc, Rc[:, :8] + float(NCELLS),
             np.zeros((SPC, 16), np.float32)], axis=1
        ).astype(np.float32)
        # R_cmp[sl, w*64 + c] = r_ext[sl, w + 8c]
        idx = (np.arange(24)[:, None] + 8 * np.arange(64)[None, :]).reshape(-1)
        r_cmp = np.ascontiguousarray(r_ext[:, idx])
        in_maps.append(
            {
                "R_ext": r_ext,
                "R_cmp": r_cmp,
                "L_A": l_a,
                "P96": p96,
                "P128": p128,
            }
        )
    return in_maps


def kernel(R, pyr_params, pyr_inv_params, fit_params, lin_W, lin_b, av, std):
    in_maps = make_inputs(
        R, pyr_params, pyr_inv_params, fit_params, lin_W, lin_b, av, std
    )
    nc = _build_program()
    res = run_bass_kernel_spmd(nc, in_maps, core_ids=list(range(NCORES)))
    return np.concatenate(
        [res.results[c]["E_out"] for c in range(NCORES)], axis=0
    ).astype(np.float32)
